# revision 25
# baseline (speedup 1.0000x reference)
"""Expert-parallel Conv1dBlock (Conv1d + GroupNorm + Mish) for Trainium2.

Strategy: 8 experts -> 8 NeuronCores, with 2-segment load balancing. The
host routes each sample to a core slot (MoE dispatch as the sharding
step). Each core runs an identical Bass/Tile program over NP sample
pairs; pairs [0, S0) use weight set w0, pairs [S0, NP) use w1. A small
DP on the host picks (NP, S0) and an expert->slot assignment so that NP
is minimal (ideal balance) while every core still applies the right
expert weights -- this beats one-expert-per-core padding to max count.

Per-core program (engine budget tuned from perfetto measurements;
TensorE is the roofline at ~40 matmuls x 119ns per sample pair):
  - conv1d as matmuls over (Cin x K) contraction in a single float32r
    pass (11-bit mantissa; conv rel err ~1e-4, far under the 2e-2 gate).
    LDWEIGHTS fully pipelines with the matmul stream, so no stationary
    reuse tricks are needed.
  - GroupNorm stats: per-channel mean/M2 via DVE bn_stats straight off
    the drained conv output, combined and group-reduced with tiny 0/1
    matmuls; rsqrt(var+eps) via the fast-inverse-sqrt bit trick +
    Newton on DVE.
  - mish(z) = z * tanh(ln(1 + e^z)): the division hides inside the ACT
    tanh table. ACT runs Exp -> Ln(x+1) -> (batched) Tanh on 2-pair
    [128x1024] tiles; Ln lives in natural_log_exp_and_others and Tanh in
    exp_and_others, and Identity/Exp are in both, so batching Tanh per
    wave costs only 2 table loads per wave. DVE does the normalize
    affine (zt) and the final z*t multiply. (The HW act2 "Mish" table
    slot emits garbage on TRN2 -- measured -- and Pool/GPSIMD elementwise
    ops run ~7.5us per instruction -- also measured -- so neither is used.)
  - stats matmuls and the deferred mish passes are split into small
    closures interleaved between the next wave's conv pairs so no engine
    queue ever blocks the PE's PSUM-drain chain.
"""

import sys

if "/opt/trn_rl_repo" not in sys.path:
    sys.path.insert(0, "/opt/trn_rl_repo")

import numpy as np

B, C, T = 512, 256, 256
E, KS, G = 8, 5, 8
EPS = 1e-5
HALF = C // 2  # 128, channels per partition block
GRP = C // G  # 32 channels per group
TP = T + 4  # padded time axis (2 halo columns each side)

PW = 8  # pairs per stats wave
MG = 4  # pairs per mish ACT batch ([128, 2048] activation tiles)
TRACE = False
LAST_EXEC_NS = None
LAST_RESULTS = None

_prog_cache = {}


def _round_f32r(x):
    """Round fp32 to the FP32R grid (1+8+11 bits, RNE) — matches walrus
    fp32_to_fp32r (downconv to 8-exp/11-mantissa)."""
    u = np.ascontiguousarray(x, dtype=np.float32).view(np.uint32)
    r = u + (0x7FF + ((u >> 12) & 1))
    r &= 0xFFFFF000
    return r.view(np.float32)


def _install_trace_hook():
    import types

    if "antenv.axon_hooks" not in sys.modules:
        mod = types.ModuleType("antenv.axon_hooks")
        holder = [None]
        mod.set_axon_ntff_profile_hook = lambda h: holder.__setitem__(0, h)
        mod.get_axon_ntff_profile_hook = lambda: holder[0]
        sys.modules["antenv.axon_hooks"] = mod
        import antenv

        antenv.axon_hooks = mod
        from trn_agent_boot.trn_boot import _ntff_profile_via_ctypes

        mod.set_axon_ntff_profile_hook(
            _ntff_profile_via_ctypes("/opt/axon/libaxon_pjrt.so")
        )
    from concourse import bass_utils

    bass_utils.upload_artifacts = lambda tmpdir: f"local:{tmpdir}"


# --------------------------------------------------------------------------
# host-side dispatch planning
# --------------------------------------------------------------------------


def _try_assign(pair_counts, S0, S1):
    """DP over experts: can counts be packed into 8 seg0 slots (cap S0)
    and 8 seg1 slots (cap S1), each slot single-expert? Returns per-expert
    (n0, n1) slot usage, or None."""
    states = {(0, 0): []}
    for c in pair_counts:
        nxt = {}
        for (u0, u1), hist in states.items():
            max_n0 = min(8 - u0, (c + S0 - 1) // S0 if S0 > 0 else 0)
            for n0 in range(0, max_n0 + 1):
                rem = c - n0 * S0
                if rem <= 0:
                    n1 = 0
                elif S1 > 0:
                    n1 = (rem + S1 - 1) // S1
                else:
                    continue
                if u1 + n1 > 8:
                    continue
                key = (u0 + n0, u1 + n1)
                if key not in nxt:
                    nxt[key] = hist + [(n0, n1)]
        states = nxt
        if not states:
            return None
    best = min(states.keys(), key=lambda k: k[0] + k[1])
    return states[best]


def _plan_segments(pair_counts):
    tot = sum(pair_counts)
    hi = max(pair_counts)
    for NP in range(max(1, (tot + 7) // 8), hi + 1):
        for S0 in range(NP, 0, -1):
            asg = _try_assign(pair_counts, S0, NP - S0)
            if asg is not None:
                return NP, S0, asg
    # always feasible: one expert per core, single segment
    return hi, hi, [(1, 0)] * len(pair_counts)


# --------------------------------------------------------------------------
# device program
# --------------------------------------------------------------------------


def _build_program(NP, S0):
    import contextlib

    import concourse.bacc as bacc
    import concourse.tile as tile
    from concourse import mybir

    dt = mybir.dt
    alu = mybir.AluOpType
    act = mybir.ActivationFunctionType

    # The act-table-load pass picks the first set serving each function, which
    # bounces Exp between exp_and_others and the Ln set on every mish batch
    # (measured: 41 table loads = 52us of ACT time). Restrict Exp to the set
    # that also holds Ln so each wave needs only the Exp/Ln<->Tanh switches.
    from concourse import hw_specs

    tabs = hw_specs.get_activation_tables("gen3")
    for name, funcs in tabs.items():
        if name != "natural_log_exp_and_others":
            funcs.discard(act.Exp)

    nc = bacc.Bacc(None, target_bir_lowering=False)

    x = nc.dram_tensor("x", [NP, 2, HALF, 2 * TP], dt.float32r, kind="ExternalInput")
    # weights laid out [co_blk, ci_blk, ci, k, co], one set per segment
    w0 = nc.dram_tensor("w0", [2, 2, HALF, KS, HALF], dt.float32r, kind="ExternalInput")
    w1 = nc.dram_tensor("w1", [2, 2, HALF, KS, HALF], dt.float32r, kind="ExternalInput")
    bias2 = nc.dram_tensor("bias2", [HALF, 2, 2], dt.float32, kind="ExternalInput")
    gamma2 = nc.dram_tensor("gamma2", [HALF, 2, 2], dt.float32, kind="ExternalInput")
    beta2 = nc.dram_tensor("beta2", [HALF, 2, 2], dt.float32, kind="ExternalInput")
    gmat = nc.dram_tensor("gmat", [2, HALF, HALF], dt.float32r, kind="ExternalInput")
    amat = nc.dram_tensor("amat", [2, HALF, HALF], dt.float32r, kind="ExternalInput")
    yo = nc.dram_tensor("yo", [NP, 2, HALF, 2, T], dt.float32, kind="ExternalOutput")

    # Declining wave sizes at the end: a wave's deferred mish work only
    # overlaps with LATER waves' conv, so the final waves must shrink or the
    # last full wave's mish becomes an un-overlapped tail (measured ~30us).
    sizes = []
    rem = NP
    while rem > PW + 2:
        sizes.append(PW)
        rem -= PW
    if rem >= 5:
        sizes.extend([(rem + 1) // 2, rem // 2])
    elif rem:
        sizes.append(rem)
    waves = []
    at = 0
    for t in sizes:
        waves.append(list(range(at, at + t)))
        at += t

    with tile.TileContext(nc) as tc:
        with contextlib.ExitStack() as ctx:
            singles = ctx.enter_context(tc.tile_pool(name="singles", bufs=1))
            xpool = ctx.enter_context(tc.tile_pool(name="xpool", bufs=8))
            cpsum = ctx.enter_context(tc.tile_pool(name="cpsum", bufs=1, space="PSUM"))
            statsum = ctx.enter_context(
                tc.tile_pool(name="statsum", bufs=1, space="PSUM"))
            ybpool = ctx.enter_context(tc.tile_pool(name="ybpool", bufs=PW))
            bnspool = ctx.enter_context(tc.tile_pool(name="bnspool", bufs=2))
            statp = ctx.enter_context(tc.tile_pool(name="statp", bufs=2))
            stp = ctx.enter_context(tc.tile_pool(name="stp", bufs=2))
            ztpool = ctx.enter_context(tc.tile_pool(name="ztpool", bufs=4))
            wpool = ctx.enter_context(tc.tile_pool(name="wpool", bufs=4))
            otpool = ctx.enter_context(tc.tile_pool(name="otpool", bufs=2))

            # ---- constants / weights resident in SBUF ----
            # One tile per (seg, cb, cib); only seg0 weights upload before the
            # first conv pair — seg1/gmat/amat DMAs are deferred behind the
            # first dgroup so they don't delay the first matmul (~10us).
            wsb = []
            for seg, wsrc in ((0, w0), (1, w1)):
                per_cb = []
                for cb in range(2):
                    per_cib = []
                    for cib in range(2):
                        wt = singles.tile([HALF, KS, HALF], dt.float32r,
                                          name=f"wsb{seg}{cb}{cib}")
                        if seg == 0:
                            nc.sync.dma_start(out=wt, in_=wsrc[cb, cib])
                        per_cib.append(wt)
                    per_cb.append(per_cib)
                wsb.append(per_cb)
            bias_s = singles.tile([HALF, 2, 2], dt.float32)
            nc.sync.dma_start(out=bias_s, in_=bias2[:, :, :])
            gamma_s = singles.tile([HALF, 2, 2], dt.float32)
            nc.sync.dma_start(out=gamma_s, in_=gamma2[:, :, :])
            beta_s = singles.tile([HALF, 2, 2], dt.float32)
            nc.sync.dma_start(out=beta_s, in_=beta2[:, :, :])
            gmat_s = singles.tile([HALF, 2, HALF], dt.float32r)
            amat_s = singles.tile([HALF, 2, HALF], dt.float32r)
            magic_s = singles.tile([G, 2 * PW], dt.int32)
            nc.vector.memset(magic_s, 0x5F3759DF)

            def emit_deferred_singles():
                for cb in range(2):
                    for cib in range(2):
                        nc.sync.dma_start(out=wsb[1][cb][cib], in_=w1[cb, cib])
                nc.sync.dma_start(out=gmat_s,
                                  in_=gmat.rearrange("c p g -> p c g"))
                nc.sync.dma_start(out=amat_s,
                                  in_=amat.rearrange("c g p -> g c p"))

            def emit_dgroup(gpairs, bns, w0p):
                """x DMA, conv matmuls, batched Identity drain + bn_stats for
                1-2 same-segment pairs sharing a 2-bank PSUM tile per cb.
                Returns [(p, iw0, (ybtile_cb0, ybtile_cb1), pi), ...]."""
                seg = 0 if gpairs[0] < S0 else 1
                xts = {}
                for p in gpairs:
                    for cib in range(2):
                        th = xpool.tile([HALF, 2, TP], dt.float32r,
                                        name=f"xh{cib}", tag=f"xh{cib}")
                        nc.sync.dma_start(out=th, in_=x[p, cib].rearrange(
                            "p (s t) -> p s t", s=2))
                        xts[(p, cib)] = th
                cps = []
                for cb in range(2):
                    cp = cpsum.tile([HALF, 2, 2, T], dt.float32,
                                    name=f"cp{cb}", tag=f"cp{cb}")
                    for pi, p in enumerate(gpairs):
                        # start=True on each pair's first matmul: clears only
                        # that pair's PSUM bank (tiles are bank-aligned).
                        first = True
                        for cib in range(2):
                            for k in range(KS):
                                for s in range(2):
                                    last = (cib == 1 and k == KS - 1 and s == 1)
                                    nc.tensor.matmul(
                                        cp[:, pi, s, :],
                                        wsb[seg][cb][cib][:, k, :],
                                        xts[(p, cib)][:, s, k:k + T],
                                        start=first, stop=last)
                                    first = False
                    cps.append(cp)
                ybs = []
                for cb in range(2):
                    yb = ybpool.tile([HALF, 2, 2, T], dt.float16,
                                     name=f"yb{cb}", tag=f"yb{cb}")
                    nyb = yb[:, :len(gpairs)]
                    nc.scalar.activation(out=nyb, in_=cps[cb][:, :len(gpairs)],
                                         func=act.Identity,
                                         bias=bias_s[:, seg, cb:cb + 1])
                    for pi, p in enumerate(gpairs):
                        iw0 = 2 * (p - w0p)
                        for s in range(2):
                            # HW restriction: BNStats output must be exactly
                            # 6 elements/partition -> one instr per sample.
                            nc.vector.bn_stats(
                                out=bns[cb][:, iw0 + s:iw0 + s + 1, :],
                                in_=yb[:, pi, s, :])
                    ybs.append(yb)
                return [(p, 2 * (p - w0p), ybs, pi)
                        for pi, p in enumerate(gpairs)]

            inv_n1 = 1.0 / (2 * GRP)   # group mean from per-channel mean/2 sums
            inv_n2 = 1.0 / (GRP * T)   # group E[y^2] from per-channel sumsq

            def build_stats(wpairs, bns, items):
                """Emit the DVE bn_stats combination now; return a list of
                closures (stats reductions, then mish chunks) to interleave
                between the next wave's conv pairs."""
                nw2 = 2 * len(wpairs)
                sp = statsum.tile([HALF, 2 * PW * 2], dt.float32, name="sp",
                                  tag="sp")
                swrs = []
                for cb in range(2):
                    bv = bns[cb]
                    swc = statp.tile([HALF, 2 * PW, 2], dt.float32,
                                     name=f"swc{cb}", tag=f"swc{cb}")
                    if nw2 < 2 * PW:
                        nc.vector.memset(swc, 0.0)
                    # S1 = mean_even + mean_odd  (= per-channel sum / 128)
                    nc.vector.tensor_tensor(out=swc[:, :nw2, 0], in0=bv[:, :nw2, 1],
                                            in1=bv[:, :nw2, 4], op=alu.add)
                    # S2 = cv_e + cv_o + 128*(m_e^2 + m_o^2)  (= chan sumsq)
                    q = statp.tile([HALF, 2 * PW], dt.float32, name="q", tag="q")
                    nc.vector.tensor_tensor(out=q[:, :nw2], in0=bv[:, :nw2, 1],
                                            in1=bv[:, :nw2, 1], op=alu.mult)
                    q2 = statp.tile([HALF, 2 * PW], dt.float32, name="q2", tag="q2")
                    nc.vector.tensor_tensor(out=q2[:, :nw2], in0=bv[:, :nw2, 4],
                                            in1=bv[:, :nw2, 4], op=alu.mult)
                    nc.vector.tensor_tensor(out=q[:, :nw2], in0=q[:, :nw2],
                                            in1=q2[:, :nw2], op=alu.add)
                    nc.vector.tensor_scalar(out=q[:, :nw2], in0=q[:, :nw2],
                                            scalar1=float(T // 2), scalar2=None,
                                            op0=alu.mult)
                    nc.vector.tensor_tensor(out=swc[:, :nw2, 1], in0=bv[:, :nw2, 2],
                                            in1=bv[:, :nw2, 5], op=alu.add)
                    nc.vector.tensor_tensor(out=swc[:, :nw2, 1],
                                            in0=swc[:, :nw2, 1], in1=q[:, :nw2],
                                            op=alu.add)
                    swr = statp.tile([HALF, 2 * PW * 2], dt.float32r,
                                     name=f"swr{cb}", tag=f"swr{cb}")
                    nc.vector.tensor_copy(swr, swc.rearrange("p a b -> p (a b)"))
                    swrs.append(swr)

                state = {}

                def c_sp():
                    nc.tensor.matmul(sp, gmat_s[:, 0, :], swrs[0],
                                     start=True, stop=False)
                    nc.tensor.matmul(sp, gmat_s[:, 1, :], swrs[1],
                                     start=False, stop=True)
                    # group stats -> -mu and rsqrt(var+eps), rows 0..G-1
                    spv = sp.rearrange("p (a b) -> p a b", b=2)
                    R = statp.tile([HALF, 2, 2 * PW], dt.float32, name="R", tag="R")
                    nc.vector.memset(R, 0.0)
                    negmu = R[0:G, 0, :nw2]
                    nc.vector.tensor_scalar(out=negmu, in0=spv[0:G, :nw2, 0],
                                            scalar1=-inv_n1, scalar2=None,
                                            op0=alu.mult)
                    m2e = statp.tile([G, 2 * PW], dt.float32, name="m2e", tag="m2e")
                    nc.vector.tensor_scalar(out=m2e[:, :nw2], in0=spv[0:G, :nw2, 1],
                                            scalar1=inv_n2, scalar2=EPS,
                                            op0=alu.mult, op1=alu.add)
                    ve = statp.tile([G, 2 * PW], dt.float32, name="ve", tag="ve")
                    nc.vector.tensor_tensor(out=ve[:, :nw2], in0=negmu, in1=negmu,
                                            op=alu.mult)
                    nc.vector.tensor_tensor(out=ve[:, :nw2], in0=m2e[:, :nw2],
                                            in1=ve[:, :nw2], op=alu.subtract)
                    # rsqrt via bit trick + Newton (all on DVE, tiny tiles)
                    yi = statp.tile([G, 2 * PW], dt.int32, name="yi", tag="yi")
                    nc.vector.tensor_scalar(out=yi[:, :nw2],
                                            in0=ve[:, :nw2].bitcast(dt.int32),
                                            scalar1=1, scalar2=None,
                                            op0=alu.arith_shift_right)
                    nc.vector.tensor_tensor(out=yi[:, :nw2], in0=magic_s[:, :nw2],
                                            in1=yi[:, :nw2], op=alu.subtract)
                    yf = yi.bitcast(dt.float32)
                    xh2 = statp.tile([G, 2 * PW], dt.float32, name="xh2", tag="xh2")
                    nc.vector.tensor_scalar(out=xh2[:, :nw2], in0=ve[:, :nw2],
                                            scalar1=0.5, scalar2=None, op0=alu.mult)
                    aa = statp.tile([G, 2 * PW], dt.float32, name="aa", tag="aa")
                    dd = statp.tile([G, 2 * PW], dt.float32, name="dd", tag="dd")
                    for it in range(3):
                        nc.vector.tensor_tensor(out=aa[:, :nw2], in0=yf[:, :nw2],
                                                in1=yf[:, :nw2], op=alu.mult)
                        nc.vector.tensor_tensor(out=aa[:, :nw2], in0=xh2[:, :nw2],
                                                in1=aa[:, :nw2], op=alu.mult)
                        nc.vector.tensor_scalar(out=dd[:, :nw2], in0=aa[:, :nw2],
                                                scalar1=-1.0, scalar2=1.5,
                                                op0=alu.mult, op1=alu.add)
                        outp = R[0:G, 1, :nw2] if it == 2 else yf[:, :nw2]
                        nc.vector.tensor_tensor(out=outp, in0=yf[:, :nw2],
                                                in1=dd[:, :nw2], op=alu.mult)
                    Rr = statp.tile([HALF, 2 * 2 * PW], dt.float32r,
                                    name="Rr", tag="Rr")
                    nc.vector.tensor_copy(Rr, R.rearrange("p a b -> p (a b)"))
                    state["Rr"] = Rr

                def seg_ranges():
                    rngs = []
                    lo = 0
                    cur = None
                    for i, p in enumerate(wpairs):
                        seg = 0 if p < S0 else 1
                        if cur is not None and seg != cur:
                            rngs.append((cur, lo, 2 * i))
                            lo = 2 * i
                        cur = seg
                    rngs.append((cur, lo, nw2))
                    return rngs

                def c_bp():
                    bpt = statsum.tile([HALF, 2, 2 * 2 * PW], dt.float32,
                                       name="bpt", tag="bpt")
                    scols = []
                    tcols = []
                    for cb in range(2):
                        nc.tensor.matmul(bpt[:, cb, :], amat_s[:, cb, :],
                                         state["Rr"], start=True, stop=True)
                        bp = bpt[:, cb, :].rearrange("p (a b) -> p a b", a=2)
                        scol = stp.tile([HALF, 2 * PW], dt.float32,
                                        name=f"scol{cb}", tag=f"scol{cb}")
                        tcol = stp.tile([HALF, 2 * PW], dt.float32,
                                        name=f"tcol{cb}", tag=f"tcol{cb}")
                        for seg, lo, hi in seg_ranges():
                            nc.vector.tensor_scalar(
                                out=scol[:, lo:hi], in0=bp[:, 1, lo:hi],
                                scalar1=gamma_s[:, seg, cb:cb + 1],
                                scalar2=None, op0=alu.mult)
                            nc.vector.tensor_tensor(
                                out=tcol[:, lo:hi], in0=bp[:, 0, lo:hi],
                                in1=scol[:, lo:hi], op=alu.mult)
                            nc.vector.tensor_scalar(
                                out=tcol[:, lo:hi], in0=tcol[:, lo:hi],
                                scalar1=beta_s[:, seg, cb:cb + 1],
                                scalar2=None, op0=alu.add)
                        scols.append(scol)
                        tcols.append(tcol)
                    state["sc"] = scols
                    state["tc"] = tcols

                mgroups = [items[i:i + MG] for i in range(0, len(items), MG)]
                els = []
                tms = []
                for mg in mgroups:
                    mg_state = {}

                    def c_expln(mg=mg, mg_state=mg_state):
                        scols, tcols = state["sc"], state["tc"]
                        zts = []
                        wts = []
                        for cb in range(2):
                            zt = ztpool.tile([HALF, MG, 2, T], dt.float16,
                                             name=f"zt{cb}", tag=f"zt{cb}")
                            for mi, (p, iw0, ybs, pi) in enumerate(mg):
                                for s in range(2):
                                    iw = iw0 + s
                                    nc.vector.tensor_scalar(
                                        out=zt[:, mi, s, :],
                                        in0=ybs[cb][:, pi, s, :],
                                        scalar1=scols[cb][:, iw:iw + 1],
                                        scalar2=tcols[cb][:, iw:iw + 1],
                                        op0=alu.mult, op1=alu.add)
                            nz = zt[:, :len(mg)]
                            wt = wpool.tile([HALF, MG, 2, T], dt.float16,
                                            name=f"wt{cb}", tag=f"wt{cb}")
                            nw = wt[:, :len(mg)]
                            nc.scalar.activation(out=nw, in_=nz, func=act.Exp)
                            nc.scalar.activation(out=nw, in_=nw, func=act.Ln,
                                                 bias=1.0)
                            zts.append(zt)
                            wts.append(wt)
                        mg_state["zt"] = zts
                        mg_state["wt"] = wts

                    def c_tanhmul(mg=mg, mg_state=mg_state):
                        for cb in range(2):
                            zt = mg_state["zt"][cb]
                            wt = mg_state["wt"][cb]
                            nw = wt[:, :len(mg)]
                            nc.scalar.activation(out=nw, in_=nw, func=act.Tanh)
                            ot = otpool.tile([HALF, MG, 2, T], dt.float32,
                                             name=f"ot{cb}", tag=f"ot{cb}")
                            nc.vector.tensor_tensor(out=ot[:, :len(mg)],
                                                    in0=zt[:, :len(mg)],
                                                    in1=nw, op=alu.mult)
                            for mi, (p, iw0, ybs, pi) in enumerate(mg):
                                nc.sync.dma_start(out=yo[p, cb], in_=ot[:, mi])

                    els.append(c_expln)
                    tms.append(c_tanhmul)

                # Exp/Ln chunks first, then all Tanh chunks back-to-back so
                # the ACT table set switches only twice per wave.
                return [c_sp, c_bp] + els + tms

            def wave_dgroups(wpairs):
                groups = []
                run = []
                run_seg = None
                for p in wpairs:
                    seg = 0 if p < S0 else 1
                    if run and (seg != run_seg or len(run) == 2):
                        groups.append(run)
                        run = []
                    run.append(p)
                    run_seg = seg
                groups.append(run)
                return groups

            pending = []
            first_dg = True
            for wpairs in waves:
                w0p = wpairs[0]
                bns = [bnspool.tile([HALF, 2 * PW, 6], dt.float32,
                                    name=f"bns{cb}", tag=f"bns{cb}")
                       for cb in range(2)]
                items = []
                # Pop schedule: c_sp/c_bp after dgroups 0/1 (their matmuls
                # need a conv block in front to hide the DVE stats chain);
                # mish closures land BEFORE later dgroups so their ACT work
                # slots into the queue ahead of drains that are still
                # waiting on conv matmuls (no ACT head-of-line blocking).
                for gi, gpairs in enumerate(wave_dgroups(wpairs)):
                    items.extend(emit_dgroup(gpairs, bns, w0p))
                    if first_dg:
                        emit_deferred_singles()
                        first_dg = False
                    for _ in range(1 if gi == 0 else 2):
                        if pending:
                            pending.pop(0)()
                while pending:
                    pending.pop(0)()
                pending = build_stats(wpairs, bns, items)
            while pending:
                pending.pop(0)()

    nc.finalize()
    return nc


# --------------------------------------------------------------------------
# host wrapper
# --------------------------------------------------------------------------


def kernel(x, use_expert_i, W, b, gamma, beta):
    global LAST_EXEC_NS, LAST_RESULTS
    from concourse.bass_utils import run_bass_kernel_spmd

    if TRACE:
        _install_trace_hook()

    x = np.asarray(x, dtype=np.float32)
    u = np.asarray(use_expert_i).astype(np.int64)
    W = np.asarray(W, dtype=np.float32)
    b = np.asarray(b, dtype=np.float32)
    gamma = np.asarray(gamma, dtype=np.float32)
    beta = np.asarray(beta, dtype=np.float32)

    idx_e = [np.nonzero(u == e)[0] for e in range(E)]
    pair_counts = [max(1, (len(ix) + 1) // 2) for ix in idx_e]
    NP, S0, asg = _plan_segments(pair_counts)

    key = (NP, S0)
    if key not in _prog_cache:
        _prog_cache[key] = _build_program(NP, S0)
    nc = _prog_cache[key]

    # ---- expert -> (core, segment) slot assembly ----
    seg_slots = [[], []]
    for e, (n0, n1) in enumerate(asg):
        seg_slots[0].extend([e] * n0)
        seg_slots[1].extend([e] * n1)
    for sl in seg_slots:
        sl.extend([-1] * (8 - len(sl)))

    # per-expert queues of (orig_index, src_index); odd counts padded
    queues = []
    for e in range(E):
        ix = idx_e[e]
        q = [(int(i), int(i)) for i in ix]
        if len(q) % 2 == 1:
            q.append((-1, int(ix[0])))
        queues.append(q)

    # group-indicator matrices, shared across cores
    gmat = np.zeros((2, HALF, HALF), np.float32)
    amat = np.zeros((2, HALF, HALF), np.float32)
    for cb in range(2):
        for p in range(HALF):
            g = cb * (G // 2) + p // GRP
            gmat[cb, p, g] = 1.0
            amat[cb, g, p] = 1.0

    def pack_w(e):
        if e < 0:
            return np.zeros((2, 2, HALF, KS, HALF), np.float32)
        we = W[e].reshape(2, HALF, 2, HALF, KS).transpose(0, 2, 3, 4, 1)
        return _round_f32r(np.ascontiguousarray(we))

    def pack_col(vec, e):
        if e < 0:
            return np.zeros((HALF, 2), np.float32)
        return np.ascontiguousarray(vec[e].reshape(2, HALF).T)

    in_maps = []
    pos_maps = []
    for core in range(8):
        pos = np.full(2 * NP, -1, np.int64)
        src = np.full(2 * NP, -1, np.int64)
        segs = (seg_slots[0][core], seg_slots[1][core])
        bounds = ((0, 2 * S0), (2 * S0, 2 * NP))
        for seg in range(2):
            e = segs[seg]
            if e < 0:
                continue
            lo, hi = bounds[seg]
            take = min(hi - lo, len(queues[e]))
            for j in range(take):
                pos[lo + j], src[lo + j] = queues[e][j]
            del queues[e][:take]

        xs = np.zeros((2 * NP, C, T), np.float32)
        valid = src >= 0
        if valid.any():
            xs[valid] = x[src[valid]]
        xpad = np.zeros((NP, 2, HALF, 2, TP), np.float32)
        xv = xs.reshape(NP, 2, 2, HALF, T).transpose(0, 2, 3, 1, 4)
        xpad[:, :, :, :, 2:2 + T] = xv

        bias_m = np.stack([pack_col(b, segs[0]), pack_col(b, segs[1])], axis=1)
        gamma_m = np.stack([pack_col(gamma, segs[0]), pack_col(gamma, segs[1])],
                           axis=1)
        beta_m = np.stack([pack_col(beta, segs[0]), pack_col(beta, segs[1])],
                          axis=1)

        in_maps.append({
            "x": _round_f32r(xpad).reshape(NP, 2, HALF, 2 * TP),
            "w0": pack_w(segs[0]),
            "w1": pack_w(segs[1]),
            "bias2": bias_m,
            "gamma2": gamma_m,
            "beta2": beta_m,
            "gmat": gmat,
            "amat": amat,
        })
        pos_maps.append(pos)

    assert all(len(q) == 0 for q in queues), "dispatch left samples unassigned"

    res = run_bass_kernel_spmd(nc, in_maps, list(range(8)), trace=TRACE)
    LAST_EXEC_NS = res.exec_time_ns
    LAST_RESULTS = res

    out = np.empty((B, C, T), np.float32)
    for core in range(8):
        pos = pos_maps[core]
        yov = res.results[core]["yo"]  # [NP, 2, 128, 2, T]
        ye = yov.transpose(0, 3, 1, 2, 4).reshape(NP * 2, C, T)
        valid = pos >= 0
        if valid.any():
            out[pos[valid]] = ye[valid]
    return out


# revision 31
# speedup vs baseline: 1.0517x; 1.0517x over previous
"""Expert-parallel Conv1dBlock (Conv1d + GroupNorm + Mish) for Trainium2.

Strategy: 8 experts -> 8 NeuronCores, with 2-segment load balancing. The
host routes each sample to a core slot (MoE dispatch as the sharding
step). Each core runs an identical Bass/Tile program over NP sample
pairs; pairs [0, S0) use weight set w0, pairs [S0, NP) use w1. A small
DP on the host picks (NP, S0) and an expert->slot assignment so that NP
is minimal (ideal balance) while every core still applies the right
expert weights -- this beats one-expert-per-core padding to max count.

Per-core program (engine budget tuned from perfetto measurements;
TensorE is the roofline at ~40 matmuls x 119ns per sample pair):
  - conv1d as matmuls over (Cin x K) contraction in a single float32r
    pass (11-bit mantissa; conv rel err ~1e-4, far under the 2e-2 gate).
    LDWEIGHTS fully pipelines with the matmul stream, so no stationary
    reuse tricks are needed.
  - GroupNorm stats: per-channel mean/M2 via DVE bn_stats straight off
    the drained conv output, combined and group-reduced with tiny 0/1
    matmuls; rsqrt(var+eps) via the fast-inverse-sqrt bit trick +
    Newton on DVE.
  - mish(z) = z * tanh(ln(1 + e^z)): the division hides inside the ACT
    tanh table. ACT runs Exp -> Ln(x+1) -> (batched) Tanh on 2-pair
    [128x1024] tiles; Ln lives in natural_log_exp_and_others and Tanh in
    exp_and_others, and Identity/Exp are in both, so batching Tanh per
    wave costs only 2 table loads per wave. DVE does the normalize
    affine (zt) and the final z*t multiply. (The HW act2 "Mish" table
    slot emits garbage on TRN2 -- measured -- and Pool/GPSIMD elementwise
    ops run ~7.5us per instruction -- also measured -- so neither is used.)
  - stats matmuls and the deferred mish passes are split into small
    closures interleaved between the next wave's conv pairs so no engine
    queue ever blocks the PE's PSUM-drain chain.
"""

import sys

if "/opt/trn_rl_repo" not in sys.path:
    sys.path.insert(0, "/opt/trn_rl_repo")

import numpy as np

B, C, T = 512, 256, 256
E, KS, G = 8, 5, 8
EPS = 1e-5
HALF = C // 2  # 128, channels per partition block
GRP = C // G  # 32 channels per group
TP = T + 4  # padded time axis (2 halo columns each side)

PW = 12  # pairs per stats wave
MG = 4  # pairs per mish ACT batch ([128, 2048] activation tiles)
TRACE = False
LAST_EXEC_NS = None
LAST_RESULTS = None

_prog_cache = {}


def _round_f32r(x):
    """Round fp32 to the FP32R grid (1+8+11 bits, RNE) — matches walrus
    fp32_to_fp32r (downconv to 8-exp/11-mantissa)."""
    u = np.ascontiguousarray(x, dtype=np.float32).view(np.uint32)
    r = u + (0x7FF + ((u >> 12) & 1))
    r &= 0xFFFFF000
    return r.view(np.float32)


def _install_trace_hook():
    import types

    if "antenv.axon_hooks" not in sys.modules:
        mod = types.ModuleType("antenv.axon_hooks")
        holder = [None]
        mod.set_axon_ntff_profile_hook = lambda h: holder.__setitem__(0, h)
        mod.get_axon_ntff_profile_hook = lambda: holder[0]
        sys.modules["antenv.axon_hooks"] = mod
        import antenv

        antenv.axon_hooks = mod
        from trn_agent_boot.trn_boot import _ntff_profile_via_ctypes

        mod.set_axon_ntff_profile_hook(
            _ntff_profile_via_ctypes("/opt/axon/libaxon_pjrt.so")
        )
    from concourse import bass_utils

    bass_utils.upload_artifacts = lambda tmpdir: f"local:{tmpdir}"


# --------------------------------------------------------------------------
# host-side dispatch planning
# --------------------------------------------------------------------------


def _try_assign(pair_counts, S0, S1):
    """DP over experts: can counts be packed into 8 seg0 slots (cap S0)
    and 8 seg1 slots (cap S1), each slot single-expert? Returns per-expert
    (n0, n1) slot usage, or None."""
    states = {(0, 0): []}
    for c in pair_counts:
        nxt = {}
        for (u0, u1), hist in states.items():
            max_n0 = min(8 - u0, (c + S0 - 1) // S0 if S0 > 0 else 0)
            for n0 in range(0, max_n0 + 1):
                rem = c - n0 * S0
                if rem <= 0:
                    n1 = 0
                elif S1 > 0:
                    n1 = (rem + S1 - 1) // S1
                else:
                    continue
                if u1 + n1 > 8:
                    continue
                key = (u0 + n0, u1 + n1)
                if key not in nxt:
                    nxt[key] = hist + [(n0, n1)]
        states = nxt
        if not states:
            return None
    best = min(states.keys(), key=lambda k: k[0] + k[1])
    return states[best]


def _plan_segments(pair_counts):
    tot = sum(pair_counts)
    hi = max(pair_counts)
    for NP in range(max(1, (tot + 7) // 8), hi + 1):
        for S0 in range(NP, 0, -1):
            asg = _try_assign(pair_counts, S0, NP - S0)
            if asg is not None:
                return NP, S0, asg
    # always feasible: one expert per core, single segment
    return hi, hi, [(1, 0)] * len(pair_counts)


# --------------------------------------------------------------------------
# device program
# --------------------------------------------------------------------------


def _build_program(NP, S0):
    import contextlib

    import concourse.bacc as bacc
    import concourse.tile as tile
    from concourse import mybir

    dt = mybir.dt
    alu = mybir.AluOpType
    act = mybir.ActivationFunctionType

    # The act-table-load pass picks the first set serving each function, which
    # bounces Exp between exp_and_others and the Ln set on every mish batch
    # (measured: 41 table loads = 52us of ACT time). Restrict Exp to the set
    # that also holds Ln so each wave needs only the Exp/Ln<->Tanh switches.
    from concourse import hw_specs

    tabs = hw_specs.get_activation_tables("gen3")
    for name, funcs in tabs.items():
        if name != "natural_log_exp_and_others":
            funcs.discard(act.Exp)

    nc = bacc.Bacc(None, target_bir_lowering=False)

    x = nc.dram_tensor("x", [NP, 2, HALF, 2 * TP], dt.float32r, kind="ExternalInput")
    # weights laid out [co_blk, ci_blk, ci, k, co], one set per segment
    w0 = nc.dram_tensor("w0", [2, 2, HALF, KS, HALF], dt.float32r, kind="ExternalInput")
    w1 = nc.dram_tensor("w1", [2, 2, HALF, KS, HALF], dt.float32r, kind="ExternalInput")
    bias2 = nc.dram_tensor("bias2", [HALF, 2, 2], dt.float32, kind="ExternalInput")
    gamma2 = nc.dram_tensor("gamma2", [HALF, 2, 2], dt.float32, kind="ExternalInput")
    beta2 = nc.dram_tensor("beta2", [HALF, 2, 2], dt.float32, kind="ExternalInput")
    gmat = nc.dram_tensor("gmat", [2, HALF, HALF], dt.float32r, kind="ExternalInput")
    amat = nc.dram_tensor("amat", [2, HALF, HALF], dt.float32r, kind="ExternalInput")
    # fp16 output: the final z*t multiply runs at 2x DVE rate with a 16-bit
    # destination and the store DMA halves; the host casts back to fp32.
    yo = nc.dram_tensor("yo", [NP, 2, HALF, 2, T], dt.float16, kind="ExternalOutput")

    # Declining wave sizes at the end: a wave's deferred mish work only
    # overlaps with LATER waves' conv, so the final waves must shrink or the
    # last full wave's mish becomes an un-overlapped tail (measured ~30us).
    sizes = []
    rem = NP
    while rem > PW + 2:
        sizes.append(PW)
        rem -= PW
    if rem >= 5:
        sizes.extend([(rem + 1) // 2, rem // 2])
    elif rem:
        sizes.append(rem)
    waves = []
    at = 0
    for t in sizes:
        waves.append(list(range(at, at + t)))
        at += t

    with tile.TileContext(nc) as tc:
        with contextlib.ExitStack() as ctx:
            singles = ctx.enter_context(tc.tile_pool(name="singles", bufs=1))
            xpool = ctx.enter_context(tc.tile_pool(name="xpool", bufs=8))
            cpsum = ctx.enter_context(tc.tile_pool(name="cpsum", bufs=1, space="PSUM"))
            statsum = ctx.enter_context(
                tc.tile_pool(name="statsum", bufs=1, space="PSUM"))
            ybpool = ctx.enter_context(tc.tile_pool(name="ybpool", bufs=PW))
            bnspool = ctx.enter_context(tc.tile_pool(name="bnspool", bufs=2))
            statp = ctx.enter_context(tc.tile_pool(name="statp", bufs=2))
            stp = ctx.enter_context(tc.tile_pool(name="stp", bufs=2))
            ztpool = ctx.enter_context(tc.tile_pool(name="ztpool", bufs=4))
            wpool = ctx.enter_context(tc.tile_pool(name="wpool", bufs=4))
            otpool = ctx.enter_context(tc.tile_pool(name="otpool", bufs=2))

            # ---- constants / weights resident in SBUF ----
            # One tile per (seg, cb, cib); only seg0 weights upload before the
            # first conv pair — seg1/gmat/amat DMAs are deferred behind the
            # first dgroup so they don't delay the first matmul (~10us).
            wsb = []
            for seg, wsrc in ((0, w0), (1, w1)):
                per_cb = []
                for cb in range(2):
                    per_cib = []
                    for cib in range(2):
                        wt = singles.tile([HALF, KS, HALF], dt.float32r,
                                          name=f"wsb{seg}{cb}{cib}")
                        if seg == 0:
                            nc.sync.dma_start(out=wt, in_=wsrc[cb, cib])
                        per_cib.append(wt)
                    per_cb.append(per_cib)
                wsb.append(per_cb)
            bias_s = singles.tile([HALF, 2, 2], dt.float32)
            nc.sync.dma_start(out=bias_s, in_=bias2[:, :, :])
            gamma_s = singles.tile([HALF, 2, 2], dt.float32)
            nc.sync.dma_start(out=gamma_s, in_=gamma2[:, :, :])
            beta_s = singles.tile([HALF, 2, 2], dt.float32)
            nc.sync.dma_start(out=beta_s, in_=beta2[:, :, :])
            gmat_s = singles.tile([HALF, 2, HALF], dt.float32r)
            amat_s = singles.tile([HALF, 2, HALF], dt.float32r)
            magic_s = singles.tile([G, 2 * PW], dt.int32)
            nc.vector.memset(magic_s, 0x5F3759DF)

            def emit_deferred_singles():
                for cb in range(2):
                    for cib in range(2):
                        nc.sync.dma_start(out=wsb[1][cb][cib], in_=w1[cb, cib])
                nc.sync.dma_start(out=gmat_s,
                                  in_=gmat.rearrange("c p g -> p c g"))
                nc.sync.dma_start(out=amat_s,
                                  in_=amat.rearrange("c g p -> g c p"))

            def emit_dgroup(gpairs, bns, w0p):
                """x DMA, conv matmuls, batched Identity drain + bn_stats for
                1-2 same-segment pairs sharing a 2-bank PSUM tile per cb.
                Returns [(p, iw0, (ybtile_cb0, ybtile_cb1), pi), ...]."""
                seg = 0 if gpairs[0] < S0 else 1
                xts = {}
                for p in gpairs:
                    for cib in range(2):
                        th = xpool.tile([HALF, 2, TP], dt.float32r,
                                        name=f"xh{cib}", tag=f"xh{cib}")
                        nc.sync.dma_start(out=th, in_=x[p, cib].rearrange(
                            "p (s t) -> p s t", s=2))
                        xts[(p, cib)] = th
                cps = []
                for cb in range(2):
                    cp = cpsum.tile([HALF, 2, 2, T], dt.float32,
                                    name=f"cp{cb}", tag=f"cp{cb}")
                    for pi, p in enumerate(gpairs):
                        # start=True on each pair's first matmul: clears only
                        # that pair's PSUM bank (tiles are bank-aligned).
                        first = True
                        for cib in range(2):
                            for k in range(KS):
                                for s in range(2):
                                    last = (cib == 1 and k == KS - 1 and s == 1)
                                    nc.tensor.matmul(
                                        cp[:, pi, s, :],
                                        wsb[seg][cb][cib][:, k, :],
                                        xts[(p, cib)][:, s, k:k + T],
                                        start=first, stop=last)
                                    first = False
                    cps.append(cp)
                ybs = []
                for cb in range(2):
                    yb = ybpool.tile([HALF, 2, 2, T], dt.float16,
                                     name=f"yb{cb}", tag=f"yb{cb}")
                    nyb = yb[:, :len(gpairs)]
                    nc.scalar.activation(out=nyb, in_=cps[cb][:, :len(gpairs)],
                                         func=act.Identity,
                                         bias=bias_s[:, seg, cb:cb + 1])
                    for pi, p in enumerate(gpairs):
                        iw0 = 2 * (p - w0p)
                        for s in range(2):
                            # HW restriction: BNStats output must be exactly
                            # 6 elements/partition -> one instr per sample.
                            nc.vector.bn_stats(
                                out=bns[cb][:, iw0 + s:iw0 + s + 1, :],
                                in_=yb[:, pi, s, :])
                    ybs.append(yb)
                return [(p, 2 * (p - w0p), ybs, pi)
                        for pi, p in enumerate(gpairs)]

            inv_n1 = 1.0 / (2 * GRP)   # group mean from per-channel mean/2 sums
            inv_n2 = 1.0 / (GRP * T)   # group E[y^2] from per-channel sumsq

            def build_stats(wpairs, bns, items):
                """Emit the DVE bn_stats combination now; return a list of
                closures (stats reductions, then mish chunks) to interleave
                between the next wave's conv pairs."""
                nw2 = 2 * len(wpairs)
                sp = statsum.tile([HALF, 2 * PW * 2], dt.float32, name="sp",
                                  tag="sp")
                swrs = []
                for cb in range(2):
                    bv = bns[cb]
                    swc = statp.tile([HALF, 2 * PW, 2], dt.float32,
                                     name=f"swc{cb}", tag=f"swc{cb}")
                    if nw2 < 2 * PW:
                        nc.vector.memset(swc, 0.0)
                    # S1 = mean_even + mean_odd  (= per-channel sum / 128)
                    nc.vector.tensor_tensor(out=swc[:, :nw2, 0], in0=bv[:, :nw2, 1],
                                            in1=bv[:, :nw2, 4], op=alu.add)
                    # S2 = cv_e + cv_o + 128*(m_e^2 + m_o^2)  (= chan sumsq)
                    q = statp.tile([HALF, 2 * PW], dt.float32, name="q", tag="q")
                    nc.vector.tensor_tensor(out=q[:, :nw2], in0=bv[:, :nw2, 1],
                                            in1=bv[:, :nw2, 1], op=alu.mult)
                    q2 = statp.tile([HALF, 2 * PW], dt.float32, name="q2", tag="q2")
                    nc.vector.tensor_tensor(out=q2[:, :nw2], in0=bv[:, :nw2, 4],
                                            in1=bv[:, :nw2, 4], op=alu.mult)
                    nc.vector.tensor_tensor(out=q[:, :nw2], in0=q[:, :nw2],
                                            in1=q2[:, :nw2], op=alu.add)
                    nc.vector.tensor_scalar(out=q[:, :nw2], in0=q[:, :nw2],
                                            scalar1=float(T // 2), scalar2=None,
                                            op0=alu.mult)
                    nc.vector.tensor_tensor(out=swc[:, :nw2, 1], in0=bv[:, :nw2, 2],
                                            in1=bv[:, :nw2, 5], op=alu.add)
                    nc.vector.tensor_tensor(out=swc[:, :nw2, 1],
                                            in0=swc[:, :nw2, 1], in1=q[:, :nw2],
                                            op=alu.add)
                    swr = statp.tile([HALF, 2 * PW * 2], dt.float32r,
                                     name=f"swr{cb}", tag=f"swr{cb}")
                    nc.vector.tensor_copy(swr, swc.rearrange("p a b -> p (a b)"))
                    swrs.append(swr)

                state = {}

                def c_sp():
                    nc.tensor.matmul(sp, gmat_s[:, 0, :], swrs[0],
                                     start=True, stop=False)
                    nc.tensor.matmul(sp, gmat_s[:, 1, :], swrs[1],
                                     start=False, stop=True)
                    # group stats -> -mu and rsqrt(var+eps), rows 0..G-1
                    spv = sp.rearrange("p (a b) -> p a b", b=2)
                    R = statp.tile([HALF, 2, 2 * PW], dt.float32, name="R", tag="R")
                    nc.vector.memset(R, 0.0)
                    negmu = R[0:G, 0, :nw2]
                    nc.vector.tensor_scalar(out=negmu, in0=spv[0:G, :nw2, 0],
                                            scalar1=-inv_n1, scalar2=None,
                                            op0=alu.mult)
                    m2e = statp.tile([G, 2 * PW], dt.float32, name="m2e", tag="m2e")
                    nc.vector.tensor_scalar(out=m2e[:, :nw2], in0=spv[0:G, :nw2, 1],
                                            scalar1=inv_n2, scalar2=EPS,
                                            op0=alu.mult, op1=alu.add)
                    ve = statp.tile([G, 2 * PW], dt.float32, name="ve", tag="ve")
                    nc.vector.tensor_tensor(out=ve[:, :nw2], in0=negmu, in1=negmu,
                                            op=alu.mult)
                    nc.vector.tensor_tensor(out=ve[:, :nw2], in0=m2e[:, :nw2],
                                            in1=ve[:, :nw2], op=alu.subtract)
                    # rsqrt via bit trick + Newton (all on DVE, tiny tiles)
                    yi = statp.tile([G, 2 * PW], dt.int32, name="yi", tag="yi")
                    nc.vector.tensor_scalar(out=yi[:, :nw2],
                                            in0=ve[:, :nw2].bitcast(dt.int32),
                                            scalar1=1, scalar2=None,
                                            op0=alu.arith_shift_right)
                    nc.vector.tensor_tensor(out=yi[:, :nw2], in0=magic_s[:, :nw2],
                                            in1=yi[:, :nw2], op=alu.subtract)
                    yf = yi.bitcast(dt.float32)
                    xh2 = statp.tile([G, 2 * PW], dt.float32, name="xh2", tag="xh2")
                    nc.vector.tensor_scalar(out=xh2[:, :nw2], in0=ve[:, :nw2],
                                            scalar1=0.5, scalar2=None, op0=alu.mult)
                    aa = statp.tile([G, 2 * PW], dt.float32, name="aa", tag="aa")
                    dd = statp.tile([G, 2 * PW], dt.float32, name="dd", tag="dd")
                    for it in range(2):
                        nc.vector.tensor_tensor(out=aa[:, :nw2], in0=yf[:, :nw2],
                                                in1=yf[:, :nw2], op=alu.mult)
                        nc.vector.tensor_tensor(out=aa[:, :nw2], in0=xh2[:, :nw2],
                                                in1=aa[:, :nw2], op=alu.mult)
                        nc.vector.tensor_scalar(out=dd[:, :nw2], in0=aa[:, :nw2],
                                                scalar1=-1.0, scalar2=1.5,
                                                op0=alu.mult, op1=alu.add)
                        outp = R[0:G, 1, :nw2] if it == 1 else yf[:, :nw2]
                        nc.vector.tensor_tensor(out=outp, in0=yf[:, :nw2],
                                                in1=dd[:, :nw2], op=alu.mult)
                    Rr = statp.tile([HALF, 2 * 2 * PW], dt.float32r,
                                    name="Rr", tag="Rr")
                    nc.vector.tensor_copy(Rr, R.rearrange("p a b -> p (a b)"))
                    state["Rr"] = Rr

                def seg_ranges():
                    rngs = []
                    lo = 0
                    cur = None
                    for i, p in enumerate(wpairs):
                        seg = 0 if p < S0 else 1
                        if cur is not None and seg != cur:
                            rngs.append((cur, lo, 2 * i))
                            lo = 2 * i
                        cur = seg
                    rngs.append((cur, lo, nw2))
                    return rngs

                def c_bp():
                    bpt = statsum.tile([HALF, 2, 2 * 2 * PW], dt.float32,
                                       name="bpt", tag="bpt")
                    scols = []
                    tcols = []
                    for cb in range(2):
                        nc.tensor.matmul(bpt[:, cb, :], amat_s[:, cb, :],
                                         state["Rr"], start=True, stop=True)
                        bp = bpt[:, cb, :].rearrange("p (a b) -> p a b", a=2)
                        scol = stp.tile([HALF, 2 * PW], dt.float32,
                                        name=f"scol{cb}", tag=f"scol{cb}")
                        tcol = stp.tile([HALF, 2 * PW], dt.float32,
                                        name=f"tcol{cb}", tag=f"tcol{cb}")
                        for seg, lo, hi in seg_ranges():
                            nc.vector.tensor_scalar(
                                out=scol[:, lo:hi], in0=bp[:, 1, lo:hi],
                                scalar1=gamma_s[:, seg, cb:cb + 1],
                                scalar2=None, op0=alu.mult)
                            nc.vector.tensor_tensor(
                                out=tcol[:, lo:hi], in0=bp[:, 0, lo:hi],
                                in1=scol[:, lo:hi], op=alu.mult)
                            nc.vector.tensor_scalar(
                                out=tcol[:, lo:hi], in0=tcol[:, lo:hi],
                                scalar1=beta_s[:, seg, cb:cb + 1],
                                scalar2=None, op0=alu.add)
                        scols.append(scol)
                        tcols.append(tcol)
                    state["sc"] = scols
                    state["tc"] = tcols

                mgroups = [items[i:i + MG] for i in range(0, len(items), MG)]
                els = []
                tms = []
                for mg in mgroups:
                    mg_state = {}

                    def c_expln(mg=mg, mg_state=mg_state):
                        scols, tcols = state["sc"], state["tc"]
                        zts = []
                        wts = []
                        for cb in range(2):
                            zt = ztpool.tile([HALF, MG, 2, T], dt.float16,
                                             name=f"zt{cb}", tag=f"zt{cb}")
                            for mi, (p, iw0, ybs, pi) in enumerate(mg):
                                for s in range(2):
                                    iw = iw0 + s
                                    nc.vector.tensor_scalar(
                                        out=zt[:, mi, s, :],
                                        in0=ybs[cb][:, pi, s, :],
                                        scalar1=scols[cb][:, iw:iw + 1],
                                        scalar2=tcols[cb][:, iw:iw + 1],
                                        op0=alu.mult, op1=alu.add)
                            nz = zt[:, :len(mg)]
                            wt = wpool.tile([HALF, MG, 2, T], dt.float16,
                                            name=f"wt{cb}", tag=f"wt{cb}")
                            nw = wt[:, :len(mg)]
                            nc.scalar.activation(out=nw, in_=nz, func=act.Exp)
                            nc.scalar.activation(out=nw, in_=nw, func=act.Ln,
                                                 bias=1.0)
                            zts.append(zt)
                            wts.append(wt)
                        mg_state["zt"] = zts
                        mg_state["wt"] = wts

                    def c_tanhmul(mg=mg, mg_state=mg_state):
                        for cb in range(2):
                            zt = mg_state["zt"][cb]
                            wt = mg_state["wt"][cb]
                            nw = wt[:, :len(mg)]
                            nc.scalar.activation(out=nw, in_=nw, func=act.Tanh)
                            ot = otpool.tile([HALF, MG, 2, T], dt.float16,
                                             name=f"ot{cb}", tag=f"ot{cb}")
                            nc.vector.tensor_tensor(out=ot[:, :len(mg)],
                                                    in0=zt[:, :len(mg)],
                                                    in1=nw, op=alu.mult)
                            for mi, (p, iw0, ybs, pi) in enumerate(mg):
                                nc.sync.dma_start(out=yo[p, cb], in_=ot[:, mi])

                    els.append(c_expln)
                    tms.append(c_tanhmul)

                # Exp/Ln chunks first, then all Tanh chunks back-to-back so
                # the ACT table set switches only twice per wave.
                return [c_sp, c_bp] + els + tms

            def wave_dgroups(wpairs):
                groups = []
                run = []
                run_seg = None
                for p in wpairs:
                    seg = 0 if p < S0 else 1
                    if run and (seg != run_seg or len(run) == 2):
                        groups.append(run)
                        run = []
                    run.append(p)
                    run_seg = seg
                groups.append(run)
                return groups

            pending = []
            first_dg = True
            for wpairs in waves:
                w0p = wpairs[0]
                bns = [bnspool.tile([HALF, 2 * PW, 6], dt.float32,
                                    name=f"bns{cb}", tag=f"bns{cb}")
                       for cb in range(2)]
                items = []
                # Pop schedule: c_sp/c_bp after dgroups 0/1 (their matmuls
                # need a conv block in front to hide the DVE stats chain);
                # mish closures land BEFORE later dgroups so their ACT work
                # slots into the queue ahead of drains that are still
                # waiting on conv matmuls (no ACT head-of-line blocking).
                for gi, gpairs in enumerate(wave_dgroups(wpairs)):
                    items.extend(emit_dgroup(gpairs, bns, w0p))
                    if first_dg:
                        emit_deferred_singles()
                        first_dg = False
                    for _ in range(1 if gi == 0 else 2):
                        if pending:
                            pending.pop(0)()
                # closures may roll across wave boundaries (small tail waves
                # drain the backlog at their own pace)
                pending.extend(build_stats(wpairs, bns, items))
            while pending:
                pending.pop(0)()

    nc.finalize()
    return nc


# --------------------------------------------------------------------------
# host wrapper
# --------------------------------------------------------------------------


def kernel(x, use_expert_i, W, b, gamma, beta):
    global LAST_EXEC_NS, LAST_RESULTS
    from concourse.bass_utils import run_bass_kernel_spmd

    if TRACE:
        _install_trace_hook()

    x = np.asarray(x, dtype=np.float32)
    u = np.asarray(use_expert_i).astype(np.int64)
    W = np.asarray(W, dtype=np.float32)
    b = np.asarray(b, dtype=np.float32)
    gamma = np.asarray(gamma, dtype=np.float32)
    beta = np.asarray(beta, dtype=np.float32)

    idx_e = [np.nonzero(u == e)[0] for e in range(E)]
    pair_counts = [max(1, (len(ix) + 1) // 2) for ix in idx_e]
    NP, S0, asg = _plan_segments(pair_counts)

    key = (NP, S0)
    if key not in _prog_cache:
        _prog_cache[key] = _build_program(NP, S0)
    nc = _prog_cache[key]

    # ---- expert -> (core, segment) slot assembly ----
    seg_slots = [[], []]
    for e, (n0, n1) in enumerate(asg):
        seg_slots[0].extend([e] * n0)
        seg_slots[1].extend([e] * n1)
    for sl in seg_slots:
        sl.extend([-1] * (8 - len(sl)))

    # per-expert queues of (orig_index, src_index); odd counts padded
    queues = []
    for e in range(E):
        ix = idx_e[e]
        q = [(int(i), int(i)) for i in ix]
        if len(q) % 2 == 1:
            q.append((-1, int(ix[0])))
        queues.append(q)

    # group-indicator matrices, shared across cores
    gmat = np.zeros((2, HALF, HALF), np.float32)
    amat = np.zeros((2, HALF, HALF), np.float32)
    for cb in range(2):
        for p in range(HALF):
            g = cb * (G // 2) + p // GRP
            gmat[cb, p, g] = 1.0
            amat[cb, g, p] = 1.0

    def pack_w(e):
        if e < 0:
            return np.zeros((2, 2, HALF, KS, HALF), np.float32)
        we = W[e].reshape(2, HALF, 2, HALF, KS).transpose(0, 2, 3, 4, 1)
        return _round_f32r(np.ascontiguousarray(we))

    def pack_col(vec, e):
        if e < 0:
            return np.zeros((HALF, 2), np.float32)
        return np.ascontiguousarray(vec[e].reshape(2, HALF).T)

    in_maps = []
    pos_maps = []
    for core in range(8):
        pos = np.full(2 * NP, -1, np.int64)
        src = np.full(2 * NP, -1, np.int64)
        segs = (seg_slots[0][core], seg_slots[1][core])
        bounds = ((0, 2 * S0), (2 * S0, 2 * NP))
        for seg in range(2):
            e = segs[seg]
            if e < 0:
                continue
            lo, hi = bounds[seg]
            take = min(hi - lo, len(queues[e]))
            for j in range(take):
                pos[lo + j], src[lo + j] = queues[e][j]
            del queues[e][:take]

        xs = np.zeros((2 * NP, C, T), np.float32)
        valid = src >= 0
        if valid.any():
            xs[valid] = x[src[valid]]
        xpad = np.zeros((NP, 2, HALF, 2, TP), np.float32)
        xv = xs.reshape(NP, 2, 2, HALF, T).transpose(0, 2, 3, 1, 4)
        xpad[:, :, :, :, 2:2 + T] = xv

        bias_m = np.stack([pack_col(b, segs[0]), pack_col(b, segs[1])], axis=1)
        gamma_m = np.stack([pack_col(gamma, segs[0]), pack_col(gamma, segs[1])],
                           axis=1)
        beta_m = np.stack([pack_col(beta, segs[0]), pack_col(beta, segs[1])],
                          axis=1)

        in_maps.append({
            "x": _round_f32r(xpad).reshape(NP, 2, HALF, 2 * TP),
            "w0": pack_w(segs[0]),
            "w1": pack_w(segs[1]),
            "bias2": bias_m,
            "gamma2": gamma_m,
            "beta2": beta_m,
            "gmat": gmat,
            "amat": amat,
        })
        pos_maps.append(pos)

    assert all(len(q) == 0 for q in queues), "dispatch left samples unassigned"

    res = run_bass_kernel_spmd(nc, in_maps, list(range(8)), trace=TRACE)
    LAST_EXEC_NS = res.exec_time_ns
    LAST_RESULTS = res

    out = np.empty((B, C, T), np.float32)
    for core in range(8):
        pos = pos_maps[core]
        yov = res.results[core]["yo"]  # [NP, 2, 128, 2, T]
        ye = yov.transpose(0, 3, 1, 2, 4).reshape(NP * 2, C, T)
        valid = pos >= 0
        if valid.any():
            out[pos[valid]] = ye[valid]
    return out
